# revision 1
# baseline (speedup 1.0000x reference)
"""Multi-head attention Bass/Tile kernel for Trainium2, 8-core SPMD.

Problem: Q,K,V [b=2, h=16, s=2048, d=64] fp32; fp16 QK^T and PV matmuls,
fp32 softmax; out fp32. 32 head-slices sharded 4-per-core across 8 cores
(pure data parallel, no collectives).

Design (~105us/core vs 151us baseline):
  - QK^T: scores psS [128 j, 2, 512 i] f32 = matmul(lhsT=KT-slice,
    rhs=QT-slice); QT/KT are [128 = 2 heads x 64 d, s] f16 pair tiles built
    by PE transposes of pool-converted loads.
  - PV uses attn as the STATIONARY operand: psO[128 i, 65] +=
    matmul(lhsT=attn[:, t, ic*128:+128], rhs=Vones[:, j, :]). Each
    accumulation step streams only d+1=65 columns (2x cheaper than
    streaming queries), the output lands [queries, d] (no epilogue
    transpose), and the ones-column accumulates the softmax denominator
    for free. The 4 i-chunk regions share one PSUM bank with a single
    start/stop accumulation group (per-element has_written semantics).
  - exp is the second bottleneck (s*s/128 rows per head on a 1 elem/cycle
    engine): split between ACT (exact exp, ~62%% of (head, i-block,
    j-group) slots) and DVE (Schraudolph int16 bit-trick exp, ~2-3%% rel
    err, EXP_DVE/EXP_TOT of slots, Bresenham-spread). Measured end-to-end
    rel err 1.4e-2 (tolerance 2e-2).
  - PV emission lags exp by PV_LAG slots so the in-order PE queue never
    blocks on an exp still in flight; i-block results are copied out of
    PSUM promptly (DVE) and normalized on the otherwise-idle GpSimd.
  - Input f32->f16 conversions run on GpSimd; K loads are issued
    front-loaded (K is fully consumed within the first i-block).
"""

import math
import os
import sys
from contextlib import ExitStack

import numpy as np

_TRN_REPO = "/opt/trn_rl_repo"
if _TRN_REPO not in sys.path:
    sys.path.insert(0, _TRN_REPO)

import concourse.bass as bass
import concourse.tile as tile
from concourse import bacc
from concourse import mybir
from concourse.bass import ds
from concourse.masks import make_identity

F32 = mybir.dt.float32
F16 = mybir.dt.float16
I16 = mybir.dt.int16

P = 128
IB = 512

SCH_SCALE = 1.4426950408889634 * 1024.0
SCH_BIAS = 15.0 * 1024.0 - 45.0

EXP_DVE = 48
EXP_TOT = 128

PV_LAG = 6


def _exp_on_dve(slot):
    return (slot * EXP_DVE) // EXP_TOT != ((slot + 1) * EXP_DVE) // EXP_TOT


def _emit_attention(tc, O_ap, Q_ap, K_ap, V_ap, per, s, d, dbg=()):
    nc = tc.nc
    dbg = set(dbg)
    ctx = ExitStack()
    scale = 1.0 / math.sqrt(d)
    SC = s // P
    NIB = s // IB
    NJG = s // (2 * P)
    NIC = IB // P
    npairs = per // 2

    consts = ctx.enter_context(tc.tile_pool(name="consts", bufs=1))
    ld32 = ctx.enter_context(tc.tile_pool(name="ld32", bufs=2))
    t16p = ctx.enter_context(tc.tile_pool(name="t16p", bufs=2))
    qkt = ctx.enter_context(tc.tile_pool(name="qkt", bufs=2))
    vps = ctx.enter_context(tc.tile_pool(name="vps", bufs=2))
    attnp = ctx.enter_context(tc.tile_pool(name="attnp", bufs=PV_LAG + 5))
    outp = ctx.enter_context(tc.tile_pool(name="outp", bufs=3))
    oaccp = ctx.enter_context(tc.tile_pool(name="oaccp", bufs=2))
    smallp = ctx.enter_context(tc.tile_pool(name="smallp", bufs=4))
    psumS = ctx.enter_context(tc.tile_pool(name="psumS", bufs=3, space="PSUM"))
    psumO = ctx.enter_context(tc.tile_pool(name="psumO", bufs=1, space="PSUM"))
    psumT = ctx.enter_context(tc.tile_pool(name="psumT", bufs=1, space="PSUM"))

    ident16 = consts.tile([P, P], F16)
    make_identity(nc, ident16)

    def pair_prologue(p, gscheds, par_issue=False):
        """Build QT/KT [128 = 2h x 64d, s] f16 for heads (2p, 2p+1).

        gsched: list of chunk-group lengths summing to SC. One DMA per
        (tensor, group) covering BOTH heads. par_issue: issue K loads from
        SP and Q loads from ACT so the first groups land concurrently
        (lead-in only — ACT is idle then). Returns (QT, KT, pieces,
        echunks)."""
        assert all(sum(gs) == SC for gs in gscheds.values())
        QT = qkt.tile([P, s], F16, tag="QT", name="QT")
        KT = qkt.tile([P, s], F16, tag="KT", name="KT")
        echunks = {"q": set(), "k": set()}
        t16s = {}
        t32s = {}
        for tname, src in (("k", K_ap), ("q", Q_ap)):
            t16s[tname] = t16p.tile([P, SC, 2 * d], F16, tag=f"t{tname}",
                                    name="t16")
            t32s[tname] = ld32.tile([P, 2, SC, d], F32, tag=f"l{tname}",
                                    name="t32")
        groups = {}
        for tname, gs in gscheds.items():
            g0 = 0
            groups[tname] = []
            for glen in gs:
                groups[tname].append((g0, glen))
                g0 += glen
        # issue order: k-g0, q-g0, then ALL remaining K groups (K is fully
        # consumed within the first i-block), then the remaining Q groups
        # K first-heavy: K is fully consumed within the first i-block,
        # Q only needs its first group until the second i-block
        order = [("k", groups["k"][0]), ("q", groups["q"][0])]
        order += [("k", g) for g in groups["k"][1:]]
        order += [("q", g) for g in groups["q"][1:]]
        for tname, (g, glen) in order:
            src = K_ap if tname == "k" else Q_ap
            srcr = src[2 * p:2 * p + 2].rearrange(
                "h (p c) d -> p h c d", p=P)
            nc.sync.dma_start(t32s[tname][:, :, g:g + glen, :],
                              srcr[:, :, g:g + glen, :])

        pieces = []
        for gi, (tname, (g, glen)) in enumerate(order):
            T_dst = KT if tname == "k" else QT
            if True:
                pass
            for hh in (0, 1):
                def conv(tname=tname, hh=hh, g=g, glen=glen):
                    nc.gpsimd.tensor_copy(
                        t16s[tname][:, g:g + glen, hh * d:(hh + 1) * d],
                        t32s[tname][:, hh, g:g + glen, :])
                pieces.append((False, conv))

            if par_issue:
                def transp_group(tname=tname, T_dst=T_dst, g=g, glen=glen):
                    pt = psumT.tile([P, glen, P], F16, tag="T", name="PT")
                    for ci in range(glen):
                        nc.tensor.transpose(pt[:, ci, :],
                                            t16s[tname][:, g + ci, :],
                                            ident16)
                    nc.vector.tensor_copy(
                        T_dst[:, g * P:(g + glen) * P], pt)
                    echunks[tname].update(range(g, g + glen))
            else:
                # prefetched pairs: xbar DMA transpose straight from t16 —
                # off the PE/DVE critical engines; latency hidden by prefetch
                def transp_group(tname=tname, T_dst=T_dst, g=g, glen=glen):
                    KT3 = T_dst.rearrange("p (c q) -> p c q", q=P)
                    nc.sync.dma_start_transpose(KT3[:, g:g + glen, :],
                                                t16s[tname][:, g:g + glen, :])
                    echunks[tname].update(range(g, g + glen))
            pieces.append((True, transp_group))
        return QT, KT, pieces, echunks

    def head_v_prologue(h):
        v32 = ld32.tile([P, SC, d], F32, tag="lv", name="v32")
        nc.sync.dma_start(v32, V_ap[h].rearrange("(p c) d -> p c d", p=P))
        Vp = vps.tile([P, SC, d + 1], F16, tag="vo", name="Vp")

        def conv(Vp=Vp, v32=v32):
            nc.gpsimd.tensor_copy(Vp[:, :, 0:d], v32)

        def ones(Vp=Vp):
            nc.gpsimd.memset(Vp[:, :, d:d + 1], 1.0)
        return Vp, [(False, conv), (False, ones)]

    # ---- emission ----
    state = {"slot": 0}
    if SC >= 16:
        GS0 = {"k": [2, 2, 4, 4, 4], "q": [4, 4, 8]}
        upfront0 = 2
    else:
        GS0 = {"k": [SC], "q": [SC]}
        upfront0 = 2
    GSN = {"k": [SC], "q": [SC]} if SC >= 8 else GS0
    QT, KT, pieces0, ech = pair_prologue(0, GS0, par_issue=True)
    Vp, vpieces = head_v_prologue(0)
    # upfront: enough piece-sets for slot 0 (K {0,1}, Q {0..3})
    upfront_n = 3 * upfront0
    for _, fn in pieces0[:upfront_n]:
        fn()
    for _, fn in vpieces:
        fn()
    prologue_pending = list(pieces0[upfront_n:])
    pv_queue = []
    cur_pair = (QT, KT)
    next_pair = None

    def drain_pv(upto_slot):
        while pv_queue and pv_queue[0][0] <= upto_slot:
            pv_queue.pop(0)[1]()

    for h in range(per):
        hh = h % 2
        p = h // 2
        QT, KT = cur_pair
        if h + 1 < per:
            Vp_next, vp_pieces = head_v_prologue(h + 1)
            prologue_pending.extend(vp_pieces)
        if hh == 0 and p + 1 < npairs:
            QT2, KT2, prol, ech2 = pair_prologue(p + 1, GSN)
            prologue_pending.extend(prol)
            next_pair = (QT2, KT2, ech2)
        hsl = ds(hh * d, d)
        for ib in range(NIB):
            isl = ds(ib * IB, IB)
            psO_box = {}
            ost = outp.tile([P, NIC, d], F32, tag="ost", name="ost")
            for jg in range(NJG):
                slot = state["slot"]
                drain_pv(slot - PV_LAG)
                budget = 3 if h == 0 else 1
                while budget > 0 and prologue_pending:
                    _, fn = prologue_pending.pop(0)
                    fn()
                    budget -= 1
                need_q = set(range(ib * NIC, (ib + 1) * NIC))
                need_k = {2 * jg, 2 * jg + 1}
                assert need_k <= ech["k"], (h, ib, jg, sorted(ech["k"]))
                assert need_q <= ech["q"], (h, ib, jg, sorted(ech["q"]))
                psS = psumS.tile([P, 2, IB], F32, tag="S", name="S")
                for t in (0, 1):
                    j = 2 * jg + t
                    jsl = ds(j * P, P)
                    nc.tensor.matmul(psS[:, t, :], KT[hsl, jsl], QT[hsl, isl],
                                     start=True, stop=True)
                a = attnp.tile([P, 2, IB], F16, tag="attn", name="attn")
                if "all_act" not in dbg and _exp_on_dve(slot):
                    nc.vector.tensor_scalar(
                        a[:].bitcast(I16), psS, SCH_SCALE * scale, SCH_BIAS,
                        mybir.AluOpType.mult, mybir.AluOpType.add)
                else:
                    nc.scalar.activation(a, psS,
                                         mybir.ActivationFunctionType.Exp,
                                         scale=scale)

                def pv(jg=jg, a=a, Vp=Vp, psO_box=psO_box, h=h, ib=ib,
                       ost=ost):
                    if jg == 0:
                        psO_box["t"] = psumO.tile([P, NIC, d + 1], F32,
                                                  tag="O", name="psO")
                    psO = psO_box["t"]
                    nmm = jg * 2 * NIC
                    last = 2 * NIC * NJG - 1
                    for t in (0, 1):
                        j = 2 * jg + t
                        for ic in range(NIC):
                            nc.tensor.matmul(
                                psO[:, ic, :], a[:, t, ic * P:(ic + 1) * P],
                                Vp[:, j, :],
                                start=(nmm == 0), stop=(nmm == last),
                                skip_group_check=True)
                            nmm += 1
                    if jg == NJG - 1:
                        oacc = oaccp.tile([P, NIC, d + 1], F32, tag="oacc",
                                          name="oacc")
                        nc.vector.tensor_copy(oacc, psO)
                        rc = smallp.tile([P, NIC, 1], F32, tag="rc", name="rc")
                        nc.vector.reciprocal(rc, oacc[:, :, d:d + 1])
                        for ic in range(NIC):
                            nc.gpsimd.tensor_scalar_mul(
                                ost[:, ic, :], oacc[:, ic, 0:d],
                                rc[:, ic, :])
                        dst = O_ap[h].rearrange("(p c) d -> p c d", p=P)
                        nc.sync.dma_start(
                            dst[:, ib * NIC:(ib + 1) * NIC, :], ost)
                pv_queue.append((slot, pv))
                state["slot"] += 1
        if hh == 1 and next_pair is not None:
            drain_pv(10 ** 9)
            for _, fn in prologue_pending:
                fn()
            prologue_pending = []
            QT2, KT2, ech = next_pair
            cur_pair = (QT2, KT2)
            next_pair = None
        if h + 1 < per:
            Vp = Vp_next
    drain_pv(10 ** 9)
    for _, fn in prologue_pending:
        fn()
    ctx.close()


def _build_nc(per, s, d, dbg=()):
    nc = bacc.Bacc()
    Qd = nc.dram_tensor("Q", [per, s, d], F32, kind="ExternalInput")
    Kd = nc.dram_tensor("K", [per, s, d], F32, kind="ExternalInput")
    Vd = nc.dram_tensor("V", [per, s, d], F32, kind="ExternalInput")
    Od = nc.dram_tensor("O", [per, s, d], F32, kind="ExternalOutput")
    with tile.TileContext(nc) as tc:
        _emit_attention(tc, Od[:], Qd[:], Kd[:], Vd[:], per, s, d, dbg=dbg)
    nc.finalize()
    return nc


_NC_CACHE = {}


def _get_nc(per, s, d):
    key = (per, s, d)
    if key not in _NC_CACHE:
        _NC_CACHE[key] = _build_nc(per, s, d)
    return _NC_CACHE[key]


N_CORES = 8


def kernel(Q, K, V):
    from concourse.bass_utils import run_bass_kernel_spmd

    Q = np.asarray(Q, dtype=np.float32)
    K = np.asarray(K, dtype=np.float32)
    V = np.asarray(V, dtype=np.float32)
    b, h, s, d = Q.shape
    bh = b * h
    per = bh // N_CORES
    Qf = np.ascontiguousarray(Q.reshape(bh, s, d))
    Kf = np.ascontiguousarray(K.reshape(bh, s, d))
    Vf = np.ascontiguousarray(V.reshape(bh, s, d))

    nc = _get_nc(per, s, d)
    in_maps = [
        {
            "Q": Qf[c * per:(c + 1) * per],
            "K": Kf[c * per:(c + 1) * per],
            "V": Vf[c * per:(c + 1) * per],
        }
        for c in range(N_CORES)
    ]
    res = run_bass_kernel_spmd(
        nc, in_maps, core_ids=list(range(N_CORES)),
        trace=bool(int(os.environ.get("KERNEL_TRACE", "0"))),
    )
    out = np.concatenate([res.results[c]["O"] for c in range(N_CORES)], axis=0)
    if bool(int(os.environ.get("KERNEL_TRACE", "0"))):
        kernel.last_results = res
    return out.reshape(b, h, s, d).astype(np.float32)

